# revision 6
# baseline (speedup 1.0000x reference)
"""Trainium2 Bass kernel for nn_BubblePredictor (LSTM B=256 T=1024 H=512 I=8).

Strategy (data-parallel over batch, 8 cores x 32 batch):
  - Recurrence layout: gates.T [2048, 32] computed as 64 bf16 matmuls/step
    (stationary = W_hh.T tiles [128,128], moving = h.T [128,32] slices).
    Gates for hidden-halves A/B go to separate PSUM banks so the elementwise
    chain for half A overlaps the matmuls for half B.
  - x-projection xg precomputed on device in 16-step blocks (K=8 matmuls),
    copied PSUM->SBUF ring, added to gates on DVE.
  - Head (logits = h @ W_head.T + b) as 4 tiny matmuls/step accumulating 16
    steps per PSUM bank; bias added during copy-out.
  - fp32 cell state; bf16 weights/activations (validated ~0.6-0.9% max rel err).
"""

import os
import time

import numpy as np
import ml_dtypes

import concourse.bass as bass
import concourse.tile as tile
from concourse import mybir
from concourse.bass import ds
from concourse.bass_utils import run_bass_kernel_spmd
from concourse.tile import add_dep_helper

BF16 = mybir.dt.bfloat16
F32 = mybir.dt.float32
AF = mybir.ActivationFunctionType
NPBF16 = ml_dtypes.bfloat16

N_CORES = 8
B_FULL, T_FULL, I_DIM, H = 256, 1024, 8, 512
B_L = B_FULL // N_CORES  # 32
G = 4 * H  # 2048
NMT = 16  # number of 128-row blocks of 4H ("m-tiles" / col-blocks)
# col-block cb -> 4H row-block. Gate order along cb: i (0:4), f (4:8), o (8:12),
# g (12:16); 4H row blocks: i=0:4, f=4:8, g=8:12, o=12:16.
MT = [0, 1, 2, 3, 4, 5, 6, 7, 12, 13, 14, 15, 8, 9, 10, 11]
XBLK = 16  # steps per xg block
LBLK = 16  # steps per logits psum bank


# ---------------------------------------------------------------------------
# Workaround for the pinned walrus build: it accepts only ONE sync-wait per
# instruction ("Too many sync wait commands"), while Tile freely attaches
# several. Engines execute in program order, so an instruction with waits
# [w1..wn] is equivalent to NoOp(w1); ...; NoOp(w_{n-1}); inst(wn). Split at
# the serialized-BIR level so every producer path is covered.
try:
    import orjson as _json_mod

    def _jloads(b):
        return _json_mod.loads(b)

    def _jdumps(d):
        return _json_mod.dumps(d)
except ImportError:  # pragma: no cover
    import json as _json_mod

    def _jloads(b):
        return _json_mod.loads(b)

    def _jdumps(d):
        return _json_mod.dumps(d).encode()


_orig_to_json_bytes = bass.Bass.to_json_bytes


def _to_json_bytes_split_waits(self):
    d = _jloads(_orig_to_json_bytes(self))
    n_extra = 0
    for fn in d.get("functions", []):
        for blk in fn.get("blocks", []):
            new_list = []
            for ins in blk.get("instructions", []):
                si = ins.get("sync_info")
                waits = (si or {}).get("on_wait") or []
                if len(waits) > 1:
                    for w in waits[:-1]:
                        n_extra += 1
                        new_list.append({
                            "debug": ins.get("debug", 0),
                            "engine": ins["engine"],
                            "ins": [],
                            "outs": [],
                            "name": f"NW-{n_extra}",
                            "opcode": "NoOp",
                            "sync_info": {"on_wait": [w], "on_update": []},
                        })
                    si["on_wait"] = [waits[-1]]
                new_list.append(ins)
            blk["instructions"] = new_list
    return _jdumps(d)


bass.Bass.to_json_bytes = _to_json_bytes_split_waits
# ---------------------------------------------------------------------------


def build(T):
    assert T % XBLK == 0 and T % LBLK == 0
    LOUT = min(64, T)  # steps per logits DMA chunk

    nc = bass.Bass("TRN2", target_bir_lowering=False, debug=False,
                   num_devices=N_CORES)
    # [p, 2048*k + 128*cb + m] = W_hh[128*MT[cb] + m, 128*k + p]
    wkm = nc.declare_dram_parameter("wkm", [128, 4 * G], BF16, isOutput=False)
    # [i, 128*cb + m] = W_ih[128*MT[cb] + m, i]
    wih = nc.declare_dram_parameter("wih", [8, G], BF16, isOutput=False)
    # [p, 2*k + c] = W_head[c, 128*k + p]
    whd = nc.declare_dram_parameter("whd", [128, 8], BF16, isOutput=False)
    bhd = nc.declare_dram_parameter("bhd", [2, 1], F32, isOutput=False)
    # [i, 32*t + b] = history[b, t, i]
    xt = nc.declare_dram_parameter("xt", [8, B_L * T], BF16, isOutput=False)
    # [c, 32*t + b] = logits[b, t, c]
    lgt = nc.declare_dram_parameter("lgt", [2, B_L * T], F32, isOutput=True)
    # [p, 32*k + b] = h_final[b, 128*k + p]
    hout = nc.declare_dram_parameter("hout", [128, 128], F32, isOutput=True)
    cout = nc.declare_dram_parameter("cout", [128, 128], F32, isOutput=True)

    with tile.TileContext(nc) as tc:
        with (
            tc.tile_pool(name="const", bufs=1) as cpool,
            tc.tile_pool(name="xg", bufs=2) as xgpool,
            tc.tile_pool(name="work", bufs=3) as wpool,
            tc.tile_pool(name="state", bufs=3) as spool,
            tc.tile_pool(name="small", bufs=3) as tpool,
            tc.tile_pool(name="lout", bufs=2) as lpool,
            tc.tile_pool(name="psum", bufs=2, space="PSUM") as pspool,
        ):
            wkm_sb = cpool.tile([128, 4 * G], BF16)
            nc.sync.dma_start(wkm_sb[:], wkm[:])
            wih_sb = cpool.tile([8, G], BF16)
            nc.sync.dma_start(wih_sb[:], wih[:])
            whd_sb = cpool.tile([128, 8], BF16)
            nc.sync.dma_start(whd_sb[:], whd[:])
            bhd_sb = cpool.tile([2, 1], F32)
            nc.sync.dma_start(bhd_sb[:], bhd[:])

            h_prev = spool.tile([128, 128], BF16, tag="h")
            c_prev = spool.tile([128, 128], F32, tag="c")
            nc.vector.memset(h_prev[:], 0.0)
            nc.vector.memset(c_prev[:], 0.0)

            xr = None
            lgp = None
            lgp_start = None
            lsb = None
            last_act = [None, None]

            for t in range(T):
                blk, tm = divmod(t, XBLK)
                if tm == 0:
                    # xg block: xg.T for steps blk*XBLK .. +XBLK
                    xch = wpool.tile([8, B_L * XBLK], BF16, tag="xch")
                    nc.sync.dma_start(
                        xch[:], xt[:, ds(B_L * XBLK * blk, B_L * XBLK)]
                    )
                    xr = xgpool.tile([128, 512 * NMT], BF16, tag="xr")
                    for cb in range(NMT):
                        xgp = pspool.tile([128, 512], F32, tag="xgp")
                        nc.tensor.matmul(
                            xgp[:],
                            wih_sb[:, ds(128 * cb, 128)],
                            xch[:],
                            start=True,
                            stop=True,
                        )
                        if cb % 2 == 0:
                            nc.vector.tensor_copy(xr[:, ds(512 * cb, 512)], xgp[:])
                        else:
                            nc.scalar.copy(xr[:, ds(512 * cb, 512)], xgp[:])

                h_new = spool.tile([128, 128], BF16, tag="h")
                c_new = spool.tile([128, 128], F32, tag="c")
                xr5 = xr[:].rearrange(
                    "p (gi j tt b) -> p gi j tt b", gi=4, j=4, tt=XBLK, b=B_L
                )

                for s in range(2):  # hidden halves
                    gp = pspool.tile([128, 256], F32, tag=f"gp{s}")
                    start_mm = None
                    for k in range(4):
                        for gi in range(4):
                            for jp in range(2):
                                cb = 4 * gi + 2 * s + jp
                                mm = nc.tensor.matmul(
                                    gp[:, ds(64 * gi + 32 * jp, 32)],
                                    wkm_sb[:, ds(2048 * k + 128 * cb, 128)],
                                    h_prev[:, ds(32 * k, 32)],
                                    start=(k == 0 and gi == 0 and jp == 0),
                                    stop=(k == 3 and gi == 3 and jp == 1),
                                    skip_group_check=True,
                                )
                                if start_mm is None:
                                    start_mm = mm.ins
                                else:
                                    add_dep_helper(
                                        mm.ins, start_mm, sync=False,
                                        reason="psum bank group order",
                                    )
                    # elementwise chain for half s
                    gs = wpool.tile([128, 256], BF16, tag=f"gs{s}")
                    gp4 = gp[:].rearrange("p (gi j b) -> p gi j b", gi=4, j=2, b=B_L)
                    gs4 = gs[:].rearrange("p (gi j b) -> p gi j b", gi=4, j=2, b=B_L)
                    nc.vector.tensor_add(
                        gs4, gp4, xr5[:, :, 2 * s : 2 * s + 2, tm, :]
                    )
                    act = wpool.tile([128, 256], BF16, tag=f"act{s}")
                    nc.scalar.activation(act[:, 0:192], gs[:, 0:192], AF.Sigmoid)
                    nc.scalar.activation(act[:, 192:256], gs[:, 192:256], AF.Tanh)
                    t2 = tpool.tile([128, 64], F32, tag=f"t2{s}")
                    nc.vector.tensor_mul(
                        t2[:], act[:, 64:128], c_prev[:, ds(64 * s, 64)]
                    )
                    t1 = tpool.tile([128, 64], BF16, tag=f"t1{s}")
                    nc.vector.tensor_mul(t1[:], act[:, 0:64], act[:, 192:256])
                    nc.vector.tensor_add(c_new[:, ds(64 * s, 64)], t1[:], t2[:])
                    th = tpool.tile([128, 64], BF16, tag=f"th{s}")
                    nc.scalar.activation(th[:], c_new[:, ds(64 * s, 64)], AF.Tanh)
                    nc.vector.tensor_mul(
                        h_new[:, ds(64 * s, 64)], act[:, 128:192], th[:]
                    )
                    last_act[s] = act

                # head: logits.T[:, 32t+b] += W_head.T chunks @ h chunks
                lt = t % LBLK
                if lt == 0:
                    lgp = pspool.tile([2, 512], F32, tag="lgp")
                    lgp_start = None
                for k in range(4):
                    mm = nc.tensor.matmul(
                        lgp[:, ds(32 * lt, 32)],
                        whd_sb[:, ds(2 * k, 2)],
                        h_new[:, ds(32 * k, 32)],
                        start=(lt == 0 and k == 0),
                        stop=(lt == LBLK - 1 and k == 3),
                        skip_group_check=True,
                    )
                    if lgp_start is None:
                        lgp_start = mm.ins
                    else:
                        add_dep_helper(
                            mm.ins, lgp_start, sync=False,
                            reason="logits bank group order",
                        )
                if lt == LBLK - 1:
                    lo = (t // LBLK) % (LOUT // LBLK)
                    if lo == 0:
                        lsb = lpool.tile([2, B_L * LOUT], F32, tag="lsb")
                    nc.vector.tensor_scalar_add(
                        lsb[:, ds(512 * lo, 512)], lgp[:], bhd_sb[:]
                    )
                    if lo == LOUT // LBLK - 1:
                        t0 = t + 1 - LOUT
                        nc.sync.dma_start(
                            lgt[:, ds(B_L * t0, B_L * LOUT)], lsb[:]
                        )

                h_prev, c_prev = h_new, c_new

            # final h in fp32 (h = o * tanh(c), fp32 tanh of fp32 c)
            thf = cpool.tile([128, 128], F32)
            nc.scalar.activation(thf[:], c_prev[:], AF.Tanh)
            hf = cpool.tile([128, 128], F32)
            for s in range(2):
                nc.vector.tensor_mul(
                    hf[:, ds(64 * s, 64)],
                    last_act[s][:, 128:192],
                    thf[:, ds(64 * s, 64)],
                )
            nc.sync.dma_start(hout[:], hf[:])
            nc.sync.dma_start(cout[:], c_prev[:])

    return nc


def _prep_weights(W_ih, W_hh, W_head, b_head):
    # wkm[p, 2048k + 128cb + m] = W_hh[128*MT[cb]+m, 128k+p]
    w4 = W_hh.reshape(NMT, 128, 4, 128)  # [mt, m, k, p]
    wkm = w4[MT].transpose(3, 2, 0, 1).reshape(128, 4 * G)  # [p, k, cb, m]
    wih = (
        W_ih.reshape(NMT, 128, I_DIM)[MT].transpose(2, 0, 1).reshape(I_DIM, G)
    )
    whd = W_head.reshape(2, 4, 128).transpose(2, 1, 0).reshape(128, 8)
    return {
        "wkm": np.ascontiguousarray(wkm).astype(NPBF16),
        "wih": np.ascontiguousarray(wih).astype(NPBF16),
        "whd": np.ascontiguousarray(whd).astype(NPBF16),
        "bhd": np.asarray(b_head, np.float32).reshape(2, 1),
    }


_RUNNER_CACHE = {}


class _Runner:
    """Compile-once executor for the SPMD kernel on 8 cores (PJRT via axon).

    Mirrors bass2jax.run_bass_via_pjrt's multi-core path but without output
    donation, so device-resident inputs/zero-buffers can be reused across
    calls for timing.
    """

    def __init__(self, T):
        import jax
        from jax.sharding import Mesh, PartitionSpec
        from jax.experimental.shard_map import shard_map
        from concourse import bass2jax

        bass2jax.install_neuronx_cc_hook()
        nc = build(T)
        self.T = T
        self.nc = nc

        in_names, out_names, out_avals = [], [], []
        partition_name = (
            nc.partition_id_tensor.name if nc.partition_id_tensor else None
        )
        for alloc in nc.m.functions[0].allocations:
            if not isinstance(alloc, mybir.MemoryLocationSet):
                continue
            name = alloc.memorylocations[0].name
            if alloc.kind == "ExternalInput":
                if name != partition_name:
                    in_names.append(name)
            elif alloc.kind == "ExternalOutput":
                shape = tuple(alloc.tensor_shape)
                dtype = mybir.dt.np(alloc.dtype)
                out_names.append(name)
                out_avals.append(jax.core.ShapedArray(shape, dtype))
        self.in_names = list(in_names)
        self.out_names = list(out_names)
        self.out_avals = out_avals
        n_params = len(in_names)
        all_in_names = in_names + out_names
        if partition_name is not None:
            all_in_names.append(partition_name)

        def _body(*args):
            operands = list(args)
            if partition_name is not None:
                operands.append(bass2jax.partition_id_tensor())
            outs = bass2jax._bass_exec_p.bind(
                *operands,
                out_avals=tuple(out_avals),
                in_names=tuple(all_in_names),
                out_names=tuple(out_names),
                lowering_input_output_aliases=(),
                sim_require_finite=True,
                sim_require_nnan=True,
                nc=nc,
            )
            return tuple(outs)

        devices = jax.devices()[: N_CORES]
        self.mesh = Mesh(np.asarray(devices), ("core",))
        n_args = n_params + len(out_names)
        self.f = jax.jit(
            shard_map(
                _body,
                mesh=self.mesh,
                in_specs=(PartitionSpec("core"),) * n_args,
                out_specs=(PartitionSpec("core"),) * len(out_names),
                check_rep=False,
            ),
            keep_unused=True,
        )
        self.zeros = [
            np.zeros((N_CORES * a.shape[0], *a.shape[1:]), a.dtype)
            for a in self.out_avals
        ]
        self._dev_args = None

    def execute(self, in_maps):
        concat_in = [
            np.concatenate([m[name] for m in in_maps], axis=0)
            for name in self.in_names
        ]
        out_arrs = self.f(*concat_in, *self.zeros)
        self._dev_args = None
        results = []
        for c in range(N_CORES):
            results.append(
                {
                    name: np.asarray(out_arrs[i]).reshape(
                        N_CORES, *self.out_avals[i].shape
                    )[c]
                    for i, name in enumerate(self.out_names)
                }
            )
        self._last_in_maps = in_maps
        return results

    def time_exec_ns(self, reps=8, trials=3):
        """Median per-execution time via wall(reps)-wall(1) deltas."""
        import jax

        in_maps = self._last_in_maps
        concat_in = [
            np.concatenate([m[name] for m in in_maps], axis=0)
            for name in self.in_names
        ]
        dev_in = [jax.device_put(a) for a in concat_in + self.zeros]

        def run_k(k):
            t0 = time.perf_counter()
            outs = []
            for _ in range(k):
                outs.append(self.f(*dev_in))
            jax.block_until_ready(outs)
            return time.perf_counter() - t0

        run_k(1)  # warm
        t1s = [run_k(1) for _ in range(trials)]
        tks = [run_k(reps) for _ in range(trials)]
        deltas = [(tk - t1) / (reps - 1) for tk, t1 in zip(tks, t1s)]
        return float(np.median(deltas) * 1e9)


def _get_runner(T):
    if T not in _RUNNER_CACHE:
        _RUNNER_CACHE[T] = _Runner(T)
    return _RUNNER_CACHE[T]


def measure_exec_ns(T=T_FULL, reps=8, trials=3):
    return _get_runner(T).time_exec_ns(reps=reps, trials=trials)


def run(history, W_ih, W_hh, W_head, b_head, T=None):
    history = np.asarray(history, np.float32)
    W_ih = np.asarray(W_ih, np.float32)
    W_hh = np.asarray(W_hh, np.float32)
    W_head = np.asarray(W_head, np.float32)
    b_head = np.asarray(b_head, np.float32)
    Bf, Tf, If = history.shape
    if T is None:
        T = Tf
    assert Bf == B_FULL and If == I_DIM

    runner = _get_runner(T)
    wmaps = _prep_weights(W_ih, W_hh, W_head, b_head)
    in_maps = []
    for c in range(N_CORES):
        hist_c = history[B_L * c : B_L * (c + 1), :T]  # [32, T, 8]
        xt = np.ascontiguousarray(hist_c.transpose(2, 1, 0)).reshape(I_DIM, T * B_L)
        in_maps.append({**wmaps, "xt": xt.astype(NPBF16)})

    results = runner.execute(in_maps)

    logits = np.empty((B_FULL, T, 2), np.float32)
    h = np.empty((B_FULL, H), np.float32)
    c_ = np.empty((B_FULL, H), np.float32)
    for c in range(N_CORES):
        r = results[c]
        sl = slice(B_L * c, B_L * (c + 1))
        logits[sl] = r["lgt"].reshape(2, T, B_L).transpose(2, 1, 0)
        h[sl] = r["hout"].reshape(128, 4, B_L).transpose(2, 1, 0).reshape(B_L, H)
        c_[sl] = r["cout"].reshape(128, 4, B_L).transpose(2, 1, 0).reshape(B_L, H)
    return logits, (h, c_)


def kernel(history, W_ih, W_hh, W_head, b_head):
    return run(history, W_ih, W_hh, W_head, b_head)
